# revision 12
# baseline (speedup 1.0000x reference)
"""Trainium2 Bass kernel for GQA causal attention block (B=2,T=2048,D=2048,H=16,G=4).

Sharding: 8 cores = batch(2) x kv-group(4). Core c handles batch b=c//4 and
kv-group g=c%4 (query heads 4g..4g+3, which share that kv group). Each core
computes a partial output y_g @ Wo[g-rows] for its batch; the host sums the 4
group partials per batch.

Per-core dataflow (all matmul inputs bf16, fp32 PSUM accumulation):
  xT  = dma-transpose(x)                    [d=128, o, t]  (contraction layouts)
  QT_h = wq_h.T @ x.T  (PE, accum over d)   [dk=128, t]
  KT   = wk.T @ x.T                         [dk=128, t]
  V    = x @ wv        (natural)            [t=128-blk, dk]
  RoPE on QT/KT via half-swap (SBUF-SBUF DMA) + mul/mul/add on DVE
  per qslice j (512 queries), head h, key block tkb<=4j+3:
    ST  = KT_blk.T-contraction QK matmul -> PSUM [tk=128, tq=512]
    PT  = exp(scale*ST) on ACT -> SBUF bf16; diag blocks masked by 0/1 mult
    yT += V_blk.T @ PT   (PE accum)          [dk=128, tq=512]
  den = ones128.T @ tree_sum(PT blocks)  (DVE sums, 1 PE matmul) [128, tq=512]
  ysb = yT * recip(den)  (DVE) -> bf16
  out[tq,:] += (partial) sum_h ysb_h.T @ wo_h  (PE accum over heads)
"""

import sys
from contextlib import ExitStack

import numpy as np

sys.path.insert(0, "/opt/trn_rl_repo")

import ml_dtypes

import bass_rust
import concourse.bass as bass
import concourse.mybir as mybir
import concourse.tile as tile
from concourse.bass_utils import run_bass_kernel_spmd

B, T, D = 2, 2048, 2048
H, G, DK = 16, 4, 128
HPC = H // G          # 4 query heads per core
P = 128
NDC = D // P          # 16 contraction chunks
NTB = T // P          # 16 token blocks
QS = 512              # query slice (matmul moving dim)
NQS = T // QS         # 4
ND = D // QS          # 4 output column slices
THETA = 10000.0
SCALE = 1.0 / float(np.sqrt(DK))
BF = mybir.dt.bfloat16
F32 = mybir.dt.float32

_CACHE = {}
_NSPLIT = [0]


def split_multi_waits(nc):
    """Walrus codegen accepts at most one sem wait per instruction; Tile's
    sem assignment can emit several. Hoist extras onto single-wait NOPs
    inserted immediately before, on the same engine stream."""
    n = 0
    for f in nc.m.functions:
        for b in f.blocks:
            insts = b.instructions
            newl = []
            changed = False
            for ins in insts:
                si = getattr(ins, "sync_info", None)
                if si is not None and si.on_wait and len(si.on_wait) > 1:
                    waits = list(si.on_wait)
                    for w in waits[:-1]:
                        _NSPLIT[0] += 1
                        nop = bass_rust.InstNoOp(
                            name=f"I-wsplit{_NSPLIT[0]}",
                            engine=ins.engine,
                            ins=[], outs=[],
                            bass_nofuse=True,
                            sync_info=mybir.SyncInfo(on_wait=[w], on_update=[]),
                        )
                        newl.append(nop)
                        n += 1
                    ins.sync_info = mybir.SyncInfo(
                        on_wait=[waits[-1]], on_update=list(si.on_update or [])
                    )
                    changed = True
                newl.append(ins)
            if changed:
                insts.clear()
                insts.extend(newl)
    return n


def build_nc():
    nc = bass.Bass()
    x = nc.declare_dram_parameter("x", [T, D], BF, isOutput=False)
    wq = nc.declare_dram_parameter("wq", [D, HPC * DK], BF, isOutput=False)
    wk = nc.declare_dram_parameter("wk", [D, DK], BF, isOutput=False)
    wv = nc.declare_dram_parameter("wv", [D, DK], BF, isOutput=False)
    wo = nc.declare_dram_parameter("wo", [HPC * DK, D], BF, isOutput=False)
    cosf = nc.declare_dram_parameter("cosf", [P, T], BF, isOutput=False)
    sinf = nc.declare_dram_parameter("sinf", [P, T], BF, isOutput=False)
    dmask = nc.declare_dram_parameter("dmask", [HPC, P, QS], BF, isOutput=False)
    out = nc.declare_dram_parameter("out", [T, D], F32, isOutput=True)

    with ExitStack() as ctx:
        tc = ctx.enter_context(tile.TileContext(nc))
        const = ctx.enter_context(tc.tile_pool(name="const", bufs=1))
        work = ctx.enter_context(tc.tile_pool(name="work", bufs=3))
        ptp = ctx.enter_context(tc.tile_pool(name="ptp", bufs=8))
        pos_ = ctx.enter_context(tc.tile_pool(name="pos_", bufs=6))
        pst = ctx.enter_context(tc.tile_pool(name="pst", bufs=3, space="PSUM"))
        pyt = ctx.enter_context(tc.tile_pool(name="pyt", bufs=2, space="PSUM"))
        pden = ctx.enter_context(tc.tile_pool(name="pden", bufs=1, space="PSUM"))
        pmm = ctx.enter_context(tc.tile_pool(name="pmm", bufs=2, space="PSUM"))

        # ---- persistent SBUF loads ----
        # Interleave per-chunk loads across the two HWDGE queues (SP carries
        # the xbar transposes, ACT the weight chunks) so the first projection
        # matmuls can start as soon as chunk 0 lands.
        xT = const.tile([P, NDC, T], BF, tag="xT")
        wq_sb = const.tile([P, NDC, HPC * DK], BF, tag="wq")
        wk_sb = const.tile([P, NDC, DK], BF, tag="wk")
        wv_sb = const.tile([P, NDC, DK], BF, tag="wv")
        wq_r = wq.rearrange("(o p) m -> p o m", p=P)
        wk_r = wk.rearrange("(o p) m -> p o m", p=P)
        wv_r = wv.rearrange("(o p) m -> p o m", p=P)
        HT = T // 2
        for o in range(NDC):
            nc.sync.dma_start(wq_sb[:, o, :], wq_r[:, o, :])
            if o == 0:
                # split the first transpose so the first projection matmul
                # (which only needs cols 0:512) can start ~0.5us earlier
                nc.sync.dma_start_transpose(
                    xT[:, o, :QS], x[:QS, o * P:(o + 1) * P])
                nc.sync.dma_start_transpose(
                    xT[:, o, QS:HT], x[QS:HT, o * P:(o + 1) * P])
            else:
                nc.sync.dma_start_transpose(
                    xT[:, o, :HT], x[:HT, o * P:(o + 1) * P])
            nc.scalar.dma_start_transpose(
                xT[:, o, HT:], x[HT:, o * P:(o + 1) * P])
            nc.scalar.dma_start(wk_sb[:, o, :], wk_r[:, o, :])
            nc.scalar.dma_start(wv_sb[:, o, :], wv_r[:, o, :])
        wo_sb = const.tile([P, HPC, D], BF, tag="wo")
        nc.scalar.dma_start(wo_sb[:], wo.rearrange("(h p) n -> p h n", p=P))
        cos_sb = const.tile([P, T], BF, tag="cos")
        nc.scalar.dma_start(cos_sb[:], cosf[:])
        sin_sb = const.tile([P, T], BF, tag="sin")
        nc.scalar.dma_start(sin_sb[:], sinf[:])
        mask_sb = const.tile([P, HPC, QS], BF, tag="mask")
        nc.scalar.dma_start(mask_sb[:], dmask.rearrange("d p q -> p d q"))
        ones_sb = const.tile([P, P], BF, tag="ones")
        nc.vector.memset(ones_sb[:], 1.0)
        # zero-init the pt pool slots: diagonal blocks only exp the unmasked
        # columns, and mask*stale-NaN would poison the sums otherwise
        for i in range(8):
            ptz = ptp.tile([P, QS], BF, tag="pt", name=f"ptz{i}")
            nc.vector.memset(ptz[:], 0.0)

        # ---- projections (sliced, interleaved with attention) ----
        # Query-slice granularity: Q/K projections + rope are emitted per
        # 512-token slice, V per 4-block quad. Slice 0 runs upfront; slice
        # j+1 is emitted inside attention j as PE filler (the attention
        # inner loop is ACT/exp throughput-bound, so the PE has idle slots).
        _pp = [(pmm, "mm"), (pst, "st"), (pyt, "yt"), (pden, "den")]
        _pg = [0]

        def proj_psum(cyc):
            if cyc:
                pool, tg = _pp[_pg[0] % 4]
                _pg[0] += 1
            else:
                pool, tg = pmm, "mm"
            return pool.tile([P, QS], F32, tag=tg, name=f"pj{_pg[0]}_{cyc}")

        def rope_slice(dst, ts, nm):  # dst: [128, 512] bf16 AP, token slice ts
            sl = slice(ts * QS, (ts + 1) * QS)
            sw = work.tile([P, QS], BF, tag="swp", name=f"sw{nm}")
            nc.gpsimd.dma_start(sw[0:64, :], dst[64:128, :])
            nc.gpsimd.dma_start(sw[64:128, :], dst[0:64, :])
            nc.vector.tensor_mul(sw[:], sw[:], sin_sb[:, sl])
            nc.vector.tensor_mul(dst, dst, cos_sb[:, sl])
            nc.vector.tensor_add(dst, dst, sw[:])

        QT = const.tile([P, HPC, T], BF, tag="QT")
        KT = const.tile([P, T], BF, tag="KT")
        Vn = const.tile([P, NTB, DK], BF, tag="Vn")

        def proj_q(h, ts, cyc=False):
            ps = proj_psum(cyc)
            for o in range(NDC):
                nc.tensor.matmul(
                    ps[:],
                    wq_sb[:, o, h * DK:(h + 1) * DK],
                    xT[:, o, ts * QS:(ts + 1) * QS],
                    start=(o == 0), stop=(o == NDC - 1),
                )
            nc.vector.tensor_copy(QT[:, h, ts * QS:(ts + 1) * QS], ps[:])

        def rope_q4(ts):
            # batched rope over all 4 heads of a query slice: one pair of
            # half-swap DMAs + 3 DVE ops on [128, 4, 512] APs (cos/sin
            # tables broadcast over the head dim with stride 0)
            sl = slice(ts * QS, (ts + 1) * QS)
            qs = QT[:, :, sl]
            sw = work.tile([P, HPC, QS], BF, tag="sw4", name=f"sw4_{ts}", bufs=2)
            nc.gpsimd.dma_start(sw[0:64, :, :], QT[64:128, :, sl])
            nc.gpsimd.dma_start(sw[64:128, :, :], QT[0:64, :, sl])
            sinb = sin_sb[:, sl].rearrange(
                "p (o c) -> p o c", o=1).broadcast_to((P, HPC, QS))
            cosb = cos_sb[:, sl].rearrange(
                "p (o c) -> p o c", o=1).broadcast_to((P, HPC, QS))
            nc.vector.tensor_mul(sw[:], sw[:], sinb)
            nc.vector.tensor_mul(qs, qs, cosb)
            nc.vector.tensor_add(qs, qs, sw[:])

        def proj_k(ts, cyc=False):
            ps = proj_psum(cyc)
            for o in range(NDC):
                nc.tensor.matmul(
                    ps[:], wk_sb[:, o, :], xT[:, o, ts * QS:(ts + 1) * QS],
                    start=(o == 0), stop=(o == NDC - 1),
                )
            nc.vector.tensor_copy(KT[:, ts * QS:(ts + 1) * QS], ps[:])
            rope_slice(KT[:, ts * QS:(ts + 1) * QS], ts, f"k{ts}")

        def proj_v_quad(jq, cyc=False):
            for tb in range(4 * jq, 4 * jq + 4):
                ps = proj_psum(cyc)
                for o in range(NDC):
                    nc.tensor.matmul(
                        ps[:, :DK], xT[:, o, tb * P:(tb + 1) * P], wv_sb[:, o, :],
                        start=(o == 0), stop=(o == NDC - 1),
                    )
                nc.vector.tensor_copy(Vn[:, tb, :], ps[:, :DK])

        for ts in range(NQS):
            for h in range(HPC):
                proj_q(h, ts, cyc=True)
            rope_q4(ts)
        for ts in range(NQS - 1):
            proj_k(ts, cyc=True)
        for jq in range(NQS - 1):
            proj_v_quad(jq, cyc=True)

        _oq = [0]

        def wo_stage(j, ysb, final=False):
            # Final stage: attention PSUM pools are free — cycle po across
            # all four to keep the PE from stalling on bank recycling, and
            # do the PSUM->SBUF copies on the (by then idle) ACT engine.
            pools = _pp if final else [(pmm, "mm")]
            gi = 0
            for tqb in range(QS // P):
                r0 = j * QS + tqb * P
                for ds in range(ND):
                    pool, tg = pools[gi % len(pools)]
                    gi += 1
                    po = pool.tile([P, QS], F32, tag=tg,
                                   name=f"po{j}_{tqb}_{ds}")
                    # narrow the last final group: pipeline copy+DMA per
                    # 128-col piece to shorten the end-of-kernel drain
                    npc = 4 if (final and tqb == QS // P - 1
                                and ds == ND - 1) else 1
                    pw = QS // npc
                    for pc in range(npc):
                        cs = slice(pc * pw, (pc + 1) * pw)
                        for h in range(HPC):
                            nc.tensor.matmul(
                                po[:, cs],
                                ysb[:, h, tqb * P:(tqb + 1) * P],
                                wo_sb[:, h, ds * QS + pc * pw:
                                      ds * QS + (pc + 1) * pw],
                                start=(h == 0), stop=(h == HPC - 1),
                            )
                        osb = pos_.tile([P, pw], F32, tag="osb",
                                        name=f"osb{j}_{tqb}_{ds}_{pc}")
                        if final and (gi + pc) % 2 == 0:
                            nc.scalar.copy(osb[:], po[:, cs])
                        else:
                            nc.vector.tensor_copy(osb[:], po[:, cs])
                        eng = nc.sync if _oq[0] % 2 == 0 else nc.gpsimd
                        _oq[0] += 1
                        eng.dma_start(
                            out[r0:r0 + P,
                                ds * QS + pc * pw:ds * QS + (pc + 1) * pw],
                            osb[:]
                        )

        ysbs = {}
        # ---- attention + output projection, per query slice ----
        for j in range(NQS):
            ysb = work.tile([P, HPC, QS], BF, tag="ysb")
            nkb = 4 * j + 4  # causal: key blocks 0..4j+3
            for h in range(HPC):
                yt = pyt.tile([P, QS], F32, tag="yt")
                den = pden.tile([P, QS], F32, tag="den")
                prev_pt = None
                ptot = None
                for tkb in range(nkb):
                    d = tkb - 4 * j
                    # columns left of 128*d are fully masked for diagonal
                    # blocks: skip them in QK/exp/AV; the mask-mult zeroes
                    # the stale region of pt so den/AV sums stay exact.
                    c0 = max(d, 0) * P
                    st = pst.tile([P, QS], F32, tag="st")
                    nc.tensor.matmul(
                        st[:, c0:],
                        KT[:, tkb * P:(tkb + 1) * P],
                        QT[:, h, j * QS + c0:(j + 1) * QS],
                        start=True, stop=True,
                    )
                    pt = ptp.tile([P, QS], BF, tag="pt")
                    nc.scalar.activation(
                        pt[:, c0:], st[:, c0:],
                        mybir.ActivationFunctionType.Exp, scale=SCALE,
                    )
                    if d >= 0:
                        nc.gpsimd.tensor_mul(pt[:], pt[:], mask_sb[:, d, :])
                    nc.tensor.matmul(
                        yt[:, c0:], Vn[:, tkb, :], pt[:, c0:],
                        start=(tkb == 0), stop=(tkb == nkb - 1),
                    )
                    # denominator: tree-sum all PT blocks on DVE, then one
                    # ones-matmul per (h, j) for the partition reduction
                    if tkb % 2 == 0:
                        prev_pt = pt
                    else:
                        pts = ptp.tile([P, QS], BF, tag="pts", name=f"pts{j}_{h}_{tkb}", bufs=4)
                        nc.vector.tensor_add(pts[:], prev_pt[:], pt[:])
                        if tkb % 4 == 1:
                            prev_pts = pts
                        else:
                            ptq = ptp.tile([P, QS], BF, tag="ptq", bufs=4,
                                           name=f"ptq{j}_{h}_{tkb}")
                            nc.vector.tensor_add(ptq[:], prev_pts[:], pts[:])
                            if ptot is None:
                                ptot = ptq
                            else:
                                nxt = ptp.tile([P, QS], BF, tag="ptt", bufs=4,
                                               name=f"ptt{j}_{h}_{tkb}")
                                nc.vector.tensor_add(nxt[:], ptot[:], ptq[:])
                                ptot = nxt
                if True:
                    nc.tensor.matmul(den[:], ones_sb[:], ptot[:],
                                     start=True, stop=True)
                recipb = work.tile([P, QS], F32, tag="recipb", name=f"rb{j}_{h}")
                nc.vector.reciprocal(recipb[:], den[:])
                nc.vector.tensor_mul(ysb[:, h, :], yt[:], recipb[:])

            ysbs[j] = ysb
            if j == 0:
                # PE filler for the exp-bound first attention slice: the
                # last K/V projections aren't needed until attention j>=3.
                proj_k(NQS - 1, cyc=False)
                proj_v_quad(NQS - 1, cyc=False)
            if j >= 1:
                wo_stage(j - 1, ysbs[j - 1])
        wo_stage(NQS - 1, ysbs[NQS - 1], final=True)
    split_multi_waits(nc)
    return nc


def _rope_tables(pos):
    inv_freq = 1.0 / (THETA ** (np.arange(0, DK // 2, dtype=np.float64) * 2.0 / DK))
    ang = pos.astype(np.float64)[:, None] * inv_freq[None, :]   # (T, 64)
    cos = np.cos(ang).T.astype(np.float32)                      # (64, T)
    sin = np.sin(ang).T.astype(np.float32)
    cosf = np.concatenate([cos, cos], axis=0)                   # (128, T)
    sinf = np.concatenate([-sin, sin], axis=0)
    return cosf, sinf


def _make_in_maps(inputs):
    x, Wq, Wk, Wv, Wo = (np.asarray(inputs[k]) for k in
                         ("x", "Wq", "Wk", "Wv", "Wo"))
    bf = ml_dtypes.bfloat16
    cosf, sinf = _rope_tables(np.asarray(inputs["pos"]))
    cosf = cosf.astype(bf)
    sinf = sinf.astype(bf)
    # diagonal-region 0/1 masks: dmask[d][tk, tq] = mask[tq, d*128 + tk]
    m = np.asarray(inputs["mask"])
    dmask = np.stack(
        [m[0:QS, d * P:(d + 1) * P].T for d in range(HPC)], axis=0
    ).astype(bf)

    in_maps = []
    for c in range(8):
        b, g = c // 4, c % 4
        in_maps.append({
            "x": x[b].astype(bf),
            "wq": Wq[:, g * HPC * DK:(g + 1) * HPC * DK].astype(bf),
            "wk": Wk[:, g * DK:(g + 1) * DK].astype(bf),
            "wv": Wv[:, g * DK:(g + 1) * DK].astype(bf),
            "wo": Wo[g * HPC * DK:(g + 1) * HPC * DK, :].astype(bf),
            "cosf": cosf, "sinf": sinf, "dmask": dmask,
        })
    return in_maps


def kernel(x, Wq, Wk, Wv, Wo, mask, pos):
    in_maps = _make_in_maps(dict(x=x, Wq=Wq, Wk=Wk, Wv=Wv, Wo=Wo,
                                 mask=mask, pos=pos))
    if "nc" not in _CACHE:
        _CACHE["nc"] = build_nc()
    nc = _CACHE["nc"]

    res = run_bass_kernel_spmd(nc, in_maps, core_ids=list(range(8)))
    outs = [r["out"] for r in res.results]
    full = np.stack([
        outs[0] + outs[1] + outs[2] + outs[3],
        outs[4] + outs[5] + outs[6] + outs[7],
    ]).astype(np.float32)
    return full



# revision 14
# speedup vs baseline: 1.0042x; 1.0042x over previous
"""Trainium2 Bass kernel for GQA causal attention block (B=2,T=2048,D=2048,H=16,G=4).

Sharding: 8 cores = batch(2) x kv-group(4). Core c handles batch b=c//4 and
kv-group g=c%4 (query heads 4g..4g+3, which share that kv group). Each core
computes a partial output y_g @ Wo[g-rows] for its batch; the host sums the 4
group partials per batch.

Per-core dataflow (all matmul inputs bf16, fp32 PSUM accumulation):
  xT  = dma-transpose(x)                    [d=128, o, t]  (contraction layouts)
  QT_h = wq_h.T @ x.T  (PE, accum over d)   [dk=128, t]
  KT   = wk.T @ x.T                         [dk=128, t]
  V    = x @ wv        (natural)            [t=128-blk, dk]
  RoPE on QT/KT via half-swap (SBUF-SBUF DMA) + mul/mul/add on DVE
  per qslice j (512 queries), head h, key block tkb<=4j+3:
    ST  = KT_blk.T-contraction QK matmul -> PSUM [tk=128, tq=512]
    PT  = exp(scale*ST) on ACT -> SBUF bf16; diag blocks masked by 0/1 mult
    yT += V_blk.T @ PT   (PE accum)          [dk=128, tq=512]
  den = ones128.T @ tree_sum(PT blocks)  (DVE sums, 1 PE matmul) [128, tq=512]
  ysb = yT * recip(den)  (DVE) -> bf16
  out[tq,:] += (partial) sum_h ysb_h.T @ wo_h  (PE accum over heads)
"""

import sys
from contextlib import ExitStack

import numpy as np

sys.path.insert(0, "/opt/trn_rl_repo")

import ml_dtypes

import bass_rust
import concourse.bass as bass
import concourse.mybir as mybir
import concourse.tile as tile
from concourse.bass_utils import run_bass_kernel_spmd

B, T, D = 2, 2048, 2048
H, G, DK = 16, 4, 128
HPC = H // G          # 4 query heads per core
P = 128
NDC = D // P          # 16 contraction chunks
NTB = T // P          # 16 token blocks
QS = 512              # query slice (matmul moving dim)
NQS = T // QS         # 4
ND = D // QS          # 4 output column slices
THETA = 10000.0
SCALE = 1.0 / float(np.sqrt(DK))
BF = mybir.dt.bfloat16
F32 = mybir.dt.float32

_CACHE = {}
_NSPLIT = [0]


def split_multi_waits(nc):
    """Walrus codegen accepts at most one sem wait per instruction; Tile's
    sem assignment can emit several. Hoist extras onto single-wait NOPs
    inserted immediately before, on the same engine stream."""
    n = 0
    for f in nc.m.functions:
        for b in f.blocks:
            insts = b.instructions
            newl = []
            changed = False
            for ins in insts:
                si = getattr(ins, "sync_info", None)
                if si is not None and si.on_wait and len(si.on_wait) > 1:
                    waits = list(si.on_wait)
                    for w in waits[:-1]:
                        _NSPLIT[0] += 1
                        nop = bass_rust.InstNoOp(
                            name=f"I-wsplit{_NSPLIT[0]}",
                            engine=ins.engine,
                            ins=[], outs=[],
                            bass_nofuse=True,
                            sync_info=mybir.SyncInfo(on_wait=[w], on_update=[]),
                        )
                        newl.append(nop)
                        n += 1
                    ins.sync_info = mybir.SyncInfo(
                        on_wait=[waits[-1]], on_update=list(si.on_update or [])
                    )
                    changed = True
                newl.append(ins)
            if changed:
                insts.clear()
                insts.extend(newl)
    return n


def build_nc():
    nc = bass.Bass()
    x = nc.declare_dram_parameter("x", [T, D], BF, isOutput=False)
    wq = nc.declare_dram_parameter("wq", [D, HPC * DK], BF, isOutput=False)
    wk = nc.declare_dram_parameter("wk", [D, DK], BF, isOutput=False)
    wv = nc.declare_dram_parameter("wv", [D, DK], BF, isOutput=False)
    wo = nc.declare_dram_parameter("wo", [HPC * DK, D], BF, isOutput=False)
    cosf = nc.declare_dram_parameter("cosf", [P, T], BF, isOutput=False)
    sinf = nc.declare_dram_parameter("sinf", [P, T], BF, isOutput=False)
    dmask = nc.declare_dram_parameter("dmask", [HPC, P, QS], BF, isOutput=False)
    out = nc.declare_dram_parameter("out", [T, D], F32, isOutput=True)

    with ExitStack() as ctx:
        tc = ctx.enter_context(tile.TileContext(nc))
        const = ctx.enter_context(tc.tile_pool(name="const", bufs=1))
        work = ctx.enter_context(tc.tile_pool(name="work", bufs=3))
        ptp = ctx.enter_context(tc.tile_pool(name="ptp", bufs=8))
        pos_ = ctx.enter_context(tc.tile_pool(name="pos_", bufs=6))
        pst = ctx.enter_context(tc.tile_pool(name="pst", bufs=3, space="PSUM"))
        pyt = ctx.enter_context(tc.tile_pool(name="pyt", bufs=2, space="PSUM"))
        pden = ctx.enter_context(tc.tile_pool(name="pden", bufs=1, space="PSUM"))
        pmm = ctx.enter_context(tc.tile_pool(name="pmm", bufs=2, space="PSUM"))

        # ---- persistent SBUF loads ----
        # Interleave per-chunk loads across the two HWDGE queues (SP carries
        # the xbar transposes, ACT the weight chunks) so the first projection
        # matmuls can start as soon as chunk 0 lands.
        xT = const.tile([P, NDC, T], BF, tag="xT")
        wq_sb = const.tile([P, NDC, HPC * DK], BF, tag="wq")
        wk_sb = const.tile([P, NDC, DK], BF, tag="wk")
        wv_sb = const.tile([P, NDC, DK], BF, tag="wv")
        wq_r = wq.rearrange("(o p) m -> p o m", p=P)
        wk_r = wk.rearrange("(o p) m -> p o m", p=P)
        wv_r = wv.rearrange("(o p) m -> p o m", p=P)
        HT = T // 2
        # first projection matmul needs wq chunk 0 + xT[:, 0, 0:512]: put
        # those two transfers at the head of DIFFERENT queues so they land
        # in parallel (~1.4us instead of ~2.4us serial)
        nc.scalar.dma_start_transpose(xT[:, 0, :QS], x[:QS, 0:P])
        for o in range(NDC):
            nc.sync.dma_start(wq_sb[:, o, :], wq_r[:, o, :])
            if o == 0:
                nc.sync.dma_start_transpose(
                    xT[:, o, QS:HT], x[QS:HT, o * P:(o + 1) * P])
            else:
                nc.sync.dma_start_transpose(
                    xT[:, o, :HT], x[:HT, o * P:(o + 1) * P])
            nc.scalar.dma_start_transpose(
                xT[:, o, HT:], x[HT:, o * P:(o + 1) * P])
            nc.scalar.dma_start(wk_sb[:, o, :], wk_r[:, o, :])
            nc.scalar.dma_start(wv_sb[:, o, :], wv_r[:, o, :])
        wo_sb = const.tile([P, HPC, D], BF, tag="wo")
        nc.scalar.dma_start(wo_sb[:], wo.rearrange("(h p) n -> p h n", p=P))
        cos_sb = const.tile([P, T], BF, tag="cos")
        nc.scalar.dma_start(cos_sb[:], cosf[:])
        sin_sb = const.tile([P, T], BF, tag="sin")
        nc.scalar.dma_start(sin_sb[:], sinf[:])
        mask_sb = const.tile([P, HPC, QS], BF, tag="mask")
        nc.scalar.dma_start(mask_sb[:], dmask.rearrange("d p q -> p d q"))
        ones_sb = const.tile([P, P], BF, tag="ones")
        nc.vector.memset(ones_sb[:], 1.0)
        # zero-init the pt pool slots: diagonal blocks only exp the unmasked
        # columns, and mask*stale-NaN would poison the sums otherwise
        for i in range(8):
            ptz = ptp.tile([P, QS], BF, tag="pt", name=f"ptz{i}")
            nc.vector.memset(ptz[:], 0.0)

        # ---- projections (sliced, interleaved with attention) ----
        # Query-slice granularity: Q/K projections + rope are emitted per
        # 512-token slice, V per 4-block quad. Slice 0 runs upfront; slice
        # j+1 is emitted inside attention j as PE filler (the attention
        # inner loop is ACT/exp throughput-bound, so the PE has idle slots).
        _pp = [(pmm, "mm"), (pst, "st"), (pyt, "yt"), (pden, "den")]
        _pg = [0]

        def proj_psum(cyc):
            if cyc:
                pool, tg = _pp[_pg[0] % 4]
                _pg[0] += 1
            else:
                pool, tg = pmm, "mm"
            return pool.tile([P, QS], F32, tag=tg, name=f"pj{_pg[0]}_{cyc}")

        def rope_slice(dst, ts, nm):  # dst: [128, 512] bf16 AP, token slice ts
            sl = slice(ts * QS, (ts + 1) * QS)
            sw = work.tile([P, QS], BF, tag="swp", name=f"sw{nm}")
            nc.gpsimd.dma_start(sw[0:64, :], dst[64:128, :])
            nc.gpsimd.dma_start(sw[64:128, :], dst[0:64, :])
            nc.vector.tensor_mul(sw[:], sw[:], sin_sb[:, sl])
            nc.vector.tensor_mul(dst, dst, cos_sb[:, sl])
            nc.vector.tensor_add(dst, dst, sw[:])

        QT = const.tile([P, HPC, T], BF, tag="QT")
        KT = const.tile([P, T], BF, tag="KT")
        Vn = const.tile([P, NTB, DK], BF, tag="Vn")

        def proj_q(h, ts, cyc=False):
            ps = proj_psum(cyc)
            for o in range(NDC):
                nc.tensor.matmul(
                    ps[:],
                    wq_sb[:, o, h * DK:(h + 1) * DK],
                    xT[:, o, ts * QS:(ts + 1) * QS],
                    start=(o == 0), stop=(o == NDC - 1),
                )
            nc.vector.tensor_copy(QT[:, h, ts * QS:(ts + 1) * QS], ps[:])

        def rope_q4(ts):
            # batched rope over all 4 heads of a query slice: one pair of
            # half-swap DMAs + 3 DVE ops on [128, 4, 512] APs (cos/sin
            # tables broadcast over the head dim with stride 0)
            sl = slice(ts * QS, (ts + 1) * QS)
            qs = QT[:, :, sl]
            sw = work.tile([P, HPC, QS], BF, tag="sw4", name=f"sw4_{ts}", bufs=2)
            nc.gpsimd.dma_start(sw[0:64, :, :], QT[64:128, :, sl])
            nc.gpsimd.dma_start(sw[64:128, :, :], QT[0:64, :, sl])
            sinb = sin_sb[:, sl].rearrange(
                "p (o c) -> p o c", o=1).broadcast_to((P, HPC, QS))
            cosb = cos_sb[:, sl].rearrange(
                "p (o c) -> p o c", o=1).broadcast_to((P, HPC, QS))
            nc.vector.tensor_mul(sw[:], sw[:], sinb)
            nc.vector.tensor_mul(qs, qs, cosb)
            nc.vector.tensor_add(qs, qs, sw[:])

        def proj_k(ts, cyc=False):
            ps = proj_psum(cyc)
            for o in range(NDC):
                nc.tensor.matmul(
                    ps[:], wk_sb[:, o, :], xT[:, o, ts * QS:(ts + 1) * QS],
                    start=(o == 0), stop=(o == NDC - 1),
                )
            nc.vector.tensor_copy(KT[:, ts * QS:(ts + 1) * QS], ps[:])
            rope_slice(KT[:, ts * QS:(ts + 1) * QS], ts, f"k{ts}")

        def proj_v_quad(jq, cyc=False):
            for tb in range(4 * jq, 4 * jq + 4):
                ps = proj_psum(cyc)
                for o in range(NDC):
                    nc.tensor.matmul(
                        ps[:, :DK], xT[:, o, tb * P:(tb + 1) * P], wv_sb[:, o, :],
                        start=(o == 0), stop=(o == NDC - 1),
                    )
                nc.vector.tensor_copy(Vn[:, tb, :], ps[:, :DK])

        for ts in range(NQS):
            for h in range(HPC):
                proj_q(h, ts, cyc=True)
            rope_q4(ts)
        for ts in range(NQS - 1):
            proj_k(ts, cyc=True)
        for jq in range(NQS - 1):
            proj_v_quad(jq, cyc=True)

        _oq = [0]

        def wo_stage(j, ysb, final=False):
            # Final stage: attention PSUM pools are free — cycle po across
            # all four to keep the PE from stalling on bank recycling, and
            # do the PSUM->SBUF copies on the (by then idle) ACT engine.
            pools = _pp if final else [(pmm, "mm")]
            gi = 0
            for tqb in range(QS // P):
                r0 = j * QS + tqb * P
                for ds in range(ND):
                    pool, tg = pools[gi % len(pools)]
                    gi += 1
                    po = pool.tile([P, QS], F32, tag=tg,
                                   name=f"po{j}_{tqb}_{ds}")
                    npc = 1
                    pw = QS // npc
                    for pc in range(npc):
                        cs = slice(pc * pw, (pc + 1) * pw)
                        for h in range(HPC):
                            nc.tensor.matmul(
                                po[:, cs],
                                ysb[:, h, tqb * P:(tqb + 1) * P],
                                wo_sb[:, h, ds * QS + pc * pw:
                                      ds * QS + (pc + 1) * pw],
                                start=(h == 0), stop=(h == HPC - 1),
                            )
                        osb = pos_.tile([P, pw], F32, tag="osb",
                                        name=f"osb{j}_{tqb}_{ds}_{pc}")
                        if final and (gi + pc) % 2 == 0:
                            nc.scalar.copy(osb[:], po[:, cs])
                        else:
                            nc.vector.tensor_copy(osb[:], po[:, cs])
                        eng = nc.sync if _oq[0] % 2 == 0 else nc.gpsimd
                        _oq[0] += 1
                        eng.dma_start(
                            out[r0:r0 + P,
                                ds * QS + pc * pw:ds * QS + (pc + 1) * pw],
                            osb[:]
                        )

        ysbs = {}
        # ---- attention + output projection, per query slice ----
        for j in range(NQS):
            ysb = work.tile([P, HPC, QS], BF, tag="ysb")
            nkb = 4 * j + 4  # causal: key blocks 0..4j+3
            for h in range(HPC):
                yt = pyt.tile([P, QS], F32, tag="yt")
                den = pden.tile([P, QS], F32, tag="den")
                prev_pt = None
                ptot = None
                for tkb in range(nkb):
                    d = tkb - 4 * j
                    # columns left of 128*d are fully masked for diagonal
                    # blocks: skip them in QK/exp/AV; the mask-mult zeroes
                    # the stale region of pt so den/AV sums stay exact.
                    c0 = max(d, 0) * P
                    st = pst.tile([P, QS], F32, tag="st")
                    nc.tensor.matmul(
                        st[:, c0:],
                        KT[:, tkb * P:(tkb + 1) * P],
                        QT[:, h, j * QS + c0:(j + 1) * QS],
                        start=True, stop=True,
                    )
                    pt = ptp.tile([P, QS], BF, tag="pt")
                    nc.scalar.activation(
                        pt[:, c0:], st[:, c0:],
                        mybir.ActivationFunctionType.Exp, scale=SCALE,
                    )
                    if d >= 0:
                        nc.gpsimd.tensor_mul(pt[:], pt[:], mask_sb[:, d, :])
                    nc.tensor.matmul(
                        yt[:, c0:], Vn[:, tkb, :], pt[:, c0:],
                        start=(tkb == 0), stop=(tkb == nkb - 1),
                    )
                    # denominator: tree-sum all PT blocks on DVE, then one
                    # ones-matmul per (h, j) for the partition reduction
                    if tkb % 2 == 0:
                        prev_pt = pt
                    else:
                        pts = ptp.tile([P, QS], BF, tag="pts", name=f"pts{j}_{h}_{tkb}", bufs=4)
                        nc.vector.tensor_add(pts[:], prev_pt[:], pt[:])
                        if tkb % 4 == 1:
                            prev_pts = pts
                        else:
                            ptq = ptp.tile([P, QS], BF, tag="ptq", bufs=4,
                                           name=f"ptq{j}_{h}_{tkb}")
                            nc.vector.tensor_add(ptq[:], prev_pts[:], pts[:])
                            if ptot is None:
                                ptot = ptq
                            else:
                                nxt = ptp.tile([P, QS], BF, tag="ptt", bufs=4,
                                               name=f"ptt{j}_{h}_{tkb}")
                                nc.vector.tensor_add(nxt[:], ptot[:], ptq[:])
                                ptot = nxt
                if True:
                    nc.tensor.matmul(den[:], ones_sb[:], ptot[:],
                                     start=True, stop=True)
                recipb = work.tile([P, QS], F32, tag="recipb", name=f"rb{j}_{h}")
                nc.vector.reciprocal(recipb[:], den[:])
                nc.vector.tensor_mul(ysb[:, h, :], yt[:], recipb[:])

            ysbs[j] = ysb
            if j == 0:
                # PE filler for the exp-bound first attention slice: the
                # last K/V projections aren't needed until attention j>=3.
                proj_k(NQS - 1, cyc=False)
                proj_v_quad(NQS - 1, cyc=False)
            if j >= 1:
                wo_stage(j - 1, ysbs[j - 1])
        wo_stage(NQS - 1, ysbs[NQS - 1], final=True)
    split_multi_waits(nc)
    return nc


def _rope_tables(pos):
    inv_freq = 1.0 / (THETA ** (np.arange(0, DK // 2, dtype=np.float64) * 2.0 / DK))
    ang = pos.astype(np.float64)[:, None] * inv_freq[None, :]   # (T, 64)
    cos = np.cos(ang).T.astype(np.float32)                      # (64, T)
    sin = np.sin(ang).T.astype(np.float32)
    cosf = np.concatenate([cos, cos], axis=0)                   # (128, T)
    sinf = np.concatenate([-sin, sin], axis=0)
    return cosf, sinf


def _make_in_maps(inputs):
    x, Wq, Wk, Wv, Wo = (np.asarray(inputs[k]) for k in
                         ("x", "Wq", "Wk", "Wv", "Wo"))
    bf = ml_dtypes.bfloat16
    cosf, sinf = _rope_tables(np.asarray(inputs["pos"]))
    cosf = cosf.astype(bf)
    sinf = sinf.astype(bf)
    # diagonal-region 0/1 masks: dmask[d][tk, tq] = mask[tq, d*128 + tk]
    m = np.asarray(inputs["mask"])
    dmask = np.stack(
        [m[0:QS, d * P:(d + 1) * P].T for d in range(HPC)], axis=0
    ).astype(bf)

    in_maps = []
    for c in range(8):
        b, g = c // 4, c % 4
        in_maps.append({
            "x": x[b].astype(bf),
            "wq": Wq[:, g * HPC * DK:(g + 1) * HPC * DK].astype(bf),
            "wk": Wk[:, g * DK:(g + 1) * DK].astype(bf),
            "wv": Wv[:, g * DK:(g + 1) * DK].astype(bf),
            "wo": Wo[g * HPC * DK:(g + 1) * HPC * DK, :].astype(bf),
            "cosf": cosf, "sinf": sinf, "dmask": dmask,
        })
    return in_maps


def kernel(x, Wq, Wk, Wv, Wo, mask, pos):
    in_maps = _make_in_maps(dict(x=x, Wq=Wq, Wk=Wk, Wv=Wv, Wo=Wo,
                                 mask=mask, pos=pos))
    if "nc" not in _CACHE:
        _CACHE["nc"] = build_nc()
    nc = _CACHE["nc"]

    res = run_bass_kernel_spmd(nc, in_maps, core_ids=list(range(8)))
    outs = [r["out"] for r in res.results]
    full = np.stack([
        outs[0] + outs[1] + outs[2] + outs[3],
        outs[4] + outs[5] + outs[6] + outs[7],
    ]).astype(np.float32)
    return full



# revision 16
# speedup vs baseline: 1.0056x; 1.0014x over previous
"""Trainium2 Bass kernel for GQA causal attention block (B=2,T=2048,D=2048,H=16,G=4).

Sharding: 8 cores = batch(2) x kv-group(4). Core c handles batch b=c//4 and
kv-group g=c%4 (query heads 4g..4g+3, which share that kv group). Each core
computes a partial output y_g @ Wo[g-rows] for its batch; the host sums the 4
group partials per batch.

Per-core dataflow (all matmul inputs bf16, fp32 PSUM accumulation):
  xT  = dma-transpose(x)                    [d=128, o, t]  (contraction layouts)
  QT_h = wq_h.T @ x.T  (PE, accum over d)   [dk=128, t]
  KT   = wk.T @ x.T                         [dk=128, t]
  V    = x @ wv        (natural)            [t=128-blk, dk]
  RoPE on QT/KT via half-swap (SBUF-SBUF DMA) + mul/mul/add on DVE
  per qslice j (512 queries), head h, key block tkb<=4j+3:
    ST  = KT_blk.T-contraction QK matmul -> PSUM [tk=128, tq=512]
    PT  = exp(scale*ST) on ACT -> SBUF bf16; diag blocks masked by 0/1 mult
    yT += V_blk.T @ PT   (PE accum)          [dk=128, tq=512]
  den = ones128.T @ tree_sum(PT blocks)  (DVE sums, 1 PE matmul) [128, tq=512]
  ysb = yT * recip(den)  (DVE) -> bf16
  out[tq,:] += (partial) sum_h ysb_h.T @ wo_h  (PE accum over heads)
"""

import sys
from contextlib import ExitStack

import numpy as np

sys.path.insert(0, "/opt/trn_rl_repo")

import ml_dtypes

import bass_rust
import concourse.bass as bass
import concourse.mybir as mybir
import concourse.tile as tile
from concourse.bass_utils import run_bass_kernel_spmd

B, T, D = 2, 2048, 2048
H, G, DK = 16, 4, 128
HPC = H // G          # 4 query heads per core
P = 128
NDC = D // P          # 16 contraction chunks
NTB = T // P          # 16 token blocks
QS = 512              # query slice (matmul moving dim)
NQS = T // QS         # 4
ND = D // QS          # 4 output column slices
THETA = 10000.0
SCALE = 1.0 / float(np.sqrt(DK))
BF = mybir.dt.bfloat16
F32 = mybir.dt.float32

_CACHE = {}
_NSPLIT = [0]


def split_multi_waits(nc):
    """Walrus codegen accepts at most one sem wait per instruction; Tile's
    sem assignment can emit several. Hoist extras onto single-wait NOPs
    inserted immediately before, on the same engine stream."""
    n = 0
    for f in nc.m.functions:
        for b in f.blocks:
            insts = b.instructions
            newl = []
            changed = False
            for ins in insts:
                si = getattr(ins, "sync_info", None)
                if si is not None and si.on_wait and len(si.on_wait) > 1:
                    waits = list(si.on_wait)
                    for w in waits[:-1]:
                        _NSPLIT[0] += 1
                        nop = bass_rust.InstNoOp(
                            name=f"I-wsplit{_NSPLIT[0]}",
                            engine=ins.engine,
                            ins=[], outs=[],
                            bass_nofuse=True,
                            sync_info=mybir.SyncInfo(on_wait=[w], on_update=[]),
                        )
                        newl.append(nop)
                        n += 1
                    ins.sync_info = mybir.SyncInfo(
                        on_wait=[waits[-1]], on_update=list(si.on_update or [])
                    )
                    changed = True
                newl.append(ins)
            if changed:
                insts.clear()
                insts.extend(newl)
    return n


def build_nc():
    nc = bass.Bass()
    x = nc.declare_dram_parameter("x", [T, D], BF, isOutput=False)
    wq = nc.declare_dram_parameter("wq", [D, HPC * DK], BF, isOutput=False)
    wk = nc.declare_dram_parameter("wk", [D, DK], BF, isOutput=False)
    wv = nc.declare_dram_parameter("wv", [D, DK], BF, isOutput=False)
    wo = nc.declare_dram_parameter("wo", [HPC * DK, D], BF, isOutput=False)
    cosf = nc.declare_dram_parameter("cosf", [P, T], BF, isOutput=False)
    sinf = nc.declare_dram_parameter("sinf", [P, T], BF, isOutput=False)
    dmask = nc.declare_dram_parameter("dmask", [HPC, P, QS], BF, isOutput=False)
    out = nc.declare_dram_parameter("out", [T, D], BF, isOutput=True)

    with ExitStack() as ctx:
        tc = ctx.enter_context(tile.TileContext(nc))
        const = ctx.enter_context(tc.tile_pool(name="const", bufs=1))
        work = ctx.enter_context(tc.tile_pool(name="work", bufs=3))
        ptp = ctx.enter_context(tc.tile_pool(name="ptp", bufs=8))
        pos_ = ctx.enter_context(tc.tile_pool(name="pos_", bufs=6))
        pst = ctx.enter_context(tc.tile_pool(name="pst", bufs=3, space="PSUM"))
        pyt = ctx.enter_context(tc.tile_pool(name="pyt", bufs=2, space="PSUM"))
        pden = ctx.enter_context(tc.tile_pool(name="pden", bufs=1, space="PSUM"))
        pmm = ctx.enter_context(tc.tile_pool(name="pmm", bufs=2, space="PSUM"))

        # ---- persistent SBUF loads ----
        # Interleave per-chunk loads across the two HWDGE queues (SP carries
        # the xbar transposes, ACT the weight chunks) so the first projection
        # matmuls can start as soon as chunk 0 lands.
        xT = const.tile([P, NDC, T], BF, tag="xT")
        wq_sb = const.tile([P, NDC, HPC * DK], BF, tag="wq")
        wk_sb = const.tile([P, NDC, DK], BF, tag="wk")
        wv_sb = const.tile([P, NDC, DK], BF, tag="wv")
        wq_r = wq.rearrange("(o p) m -> p o m", p=P)
        wk_r = wk.rearrange("(o p) m -> p o m", p=P)
        wv_r = wv.rearrange("(o p) m -> p o m", p=P)
        HT = T // 2
        # first projection matmul needs wq chunk 0 + xT[:, 0, 0:512]: put
        # those two transfers at the head of DIFFERENT queues so they land
        # in parallel (~1.4us instead of ~2.4us serial)
        nc.scalar.dma_start_transpose(xT[:, 0, :QS], x[:QS, 0:P])
        for o in range(NDC):
            nc.sync.dma_start(wq_sb[:, o, :], wq_r[:, o, :])
            if o == 0:
                nc.sync.dma_start_transpose(
                    xT[:, o, QS:HT], x[QS:HT, o * P:(o + 1) * P])
            else:
                nc.sync.dma_start_transpose(
                    xT[:, o, :HT], x[:HT, o * P:(o + 1) * P])
            nc.scalar.dma_start_transpose(
                xT[:, o, HT:], x[HT:, o * P:(o + 1) * P])
            nc.scalar.dma_start(wk_sb[:, o, :], wk_r[:, o, :])
            nc.scalar.dma_start(wv_sb[:, o, :], wv_r[:, o, :])
        wo_sb = const.tile([P, HPC, D], BF, tag="wo")
        nc.scalar.dma_start(wo_sb[:], wo.rearrange("(h p) n -> p h n", p=P))
        cos_sb = const.tile([P, T], BF, tag="cos")
        nc.scalar.dma_start(cos_sb[:], cosf[:])
        sin_sb = const.tile([P, T], BF, tag="sin")
        nc.scalar.dma_start(sin_sb[:], sinf[:])
        mask_sb = const.tile([P, HPC, QS], BF, tag="mask")
        nc.scalar.dma_start(mask_sb[:], dmask.rearrange("d p q -> p d q"))
        ones_sb = const.tile([P, P], BF, tag="ones")
        nc.vector.memset(ones_sb[:], 1.0)
        # zero-init the pt pool slots: diagonal blocks only exp the unmasked
        # columns, and mask*stale-NaN would poison the sums otherwise
        for i in range(8):
            ptz = ptp.tile([P, QS], BF, tag="pt", name=f"ptz{i}")
            nc.vector.memset(ptz[:], 0.0)

        # ---- projections (sliced, interleaved with attention) ----
        # Query-slice granularity: Q/K projections + rope are emitted per
        # 512-token slice, V per 4-block quad. Slice 0 runs upfront; slice
        # j+1 is emitted inside attention j as PE filler (the attention
        # inner loop is ACT/exp throughput-bound, so the PE has idle slots).
        _pp = [(pmm, "mm"), (pst, "st"), (pyt, "yt"), (pden, "den")]
        _pg = [0]

        def proj_psum(cyc):
            if cyc:
                pool, tg = _pp[_pg[0] % 4]
                _pg[0] += 1
            else:
                pool, tg = pmm, "mm"
            return pool.tile([P, QS], F32, tag=tg, name=f"pj{_pg[0]}_{cyc}")

        def rope_slice(dst, ts, nm):  # dst: [128, 512] bf16 AP, token slice ts
            sl = slice(ts * QS, (ts + 1) * QS)
            sw = work.tile([P, QS], BF, tag="swp", name=f"sw{nm}")
            nc.gpsimd.dma_start(sw[0:64, :], dst[64:128, :])
            nc.gpsimd.dma_start(sw[64:128, :], dst[0:64, :])
            nc.vector.tensor_mul(sw[:], sw[:], sin_sb[:, sl])
            nc.vector.tensor_mul(dst, dst, cos_sb[:, sl])
            nc.vector.tensor_add(dst, dst, sw[:])

        QT = const.tile([P, HPC, T], BF, tag="QT")
        KT = const.tile([P, T], BF, tag="KT")
        Vn = const.tile([P, NTB, DK], BF, tag="Vn")

        def proj_q(h, ts, cyc=False):
            ps = proj_psum(cyc)
            for o in range(NDC):
                nc.tensor.matmul(
                    ps[:],
                    wq_sb[:, o, h * DK:(h + 1) * DK],
                    xT[:, o, ts * QS:(ts + 1) * QS],
                    start=(o == 0), stop=(o == NDC - 1),
                )
            nc.vector.tensor_copy(QT[:, h, ts * QS:(ts + 1) * QS], ps[:])

        def rope_q4(ts):
            # batched rope over all 4 heads of a query slice: one pair of
            # half-swap DMAs + 3 DVE ops on [128, 4, 512] APs (cos/sin
            # tables broadcast over the head dim with stride 0)
            sl = slice(ts * QS, (ts + 1) * QS)
            qs = QT[:, :, sl]
            sw = work.tile([P, HPC, QS], BF, tag="sw4", name=f"sw4_{ts}", bufs=2)
            nc.gpsimd.dma_start(sw[0:64, :, :], QT[64:128, :, sl])
            nc.gpsimd.dma_start(sw[64:128, :, :], QT[0:64, :, sl])
            sinb = sin_sb[:, sl].rearrange(
                "p (o c) -> p o c", o=1).broadcast_to((P, HPC, QS))
            cosb = cos_sb[:, sl].rearrange(
                "p (o c) -> p o c", o=1).broadcast_to((P, HPC, QS))
            nc.vector.tensor_mul(sw[:], sw[:], sinb)
            nc.vector.tensor_mul(qs, qs, cosb)
            nc.vector.tensor_add(qs, qs, sw[:])

        def proj_k(ts, cyc=False):
            ps = proj_psum(cyc)
            for o in range(NDC):
                nc.tensor.matmul(
                    ps[:], wk_sb[:, o, :], xT[:, o, ts * QS:(ts + 1) * QS],
                    start=(o == 0), stop=(o == NDC - 1),
                )
            nc.vector.tensor_copy(KT[:, ts * QS:(ts + 1) * QS], ps[:])
            rope_slice(KT[:, ts * QS:(ts + 1) * QS], ts, f"k{ts}")

        def proj_v_quad(jq, cyc=False):
            for tb in range(4 * jq, 4 * jq + 4):
                ps = proj_psum(cyc)
                for o in range(NDC):
                    nc.tensor.matmul(
                        ps[:, :DK], xT[:, o, tb * P:(tb + 1) * P], wv_sb[:, o, :],
                        start=(o == 0), stop=(o == NDC - 1),
                    )
                nc.vector.tensor_copy(Vn[:, tb, :], ps[:, :DK])

        for ts in range(NQS):
            for h in range(HPC):
                proj_q(h, ts, cyc=True)
            rope_q4(ts)
        for ts in range(NQS - 1):
            proj_k(ts, cyc=True)
        for jq in range(NQS - 1):
            proj_v_quad(jq, cyc=True)

        _oq = [0]

        def wo_stage(j, ysb, final=False):
            # Final stage: attention PSUM pools are free — cycle po across
            # all four to keep the PE from stalling on bank recycling, and
            # do the PSUM->SBUF copies on the (by then idle) ACT engine.
            pools = _pp if final else [(pmm, "mm")]
            gi = 0
            for tqb in range(QS // P):
                r0 = j * QS + tqb * P
                for ds in range(ND):
                    pool, tg = pools[gi % len(pools)]
                    gi += 1
                    po = pool.tile([P, QS], F32, tag=tg,
                                   name=f"po{j}_{tqb}_{ds}")
                    npc = 1
                    pw = QS // npc
                    for pc in range(npc):
                        cs = slice(pc * pw, (pc + 1) * pw)
                        for h in range(HPC):
                            nc.tensor.matmul(
                                po[:, cs],
                                ysb[:, h, tqb * P:(tqb + 1) * P],
                                wo_sb[:, h, ds * QS + pc * pw:
                                      ds * QS + (pc + 1) * pw],
                                start=(h == 0), stop=(h == HPC - 1),
                            )
                        osb = pos_.tile([P, pw], BF, tag="osb",
                                        name=f"osb{j}_{tqb}_{ds}_{pc}")
                        if final and (gi + pc) % 2 == 0:
                            nc.scalar.copy(osb[:], po[:, cs])
                        else:
                            nc.vector.tensor_copy(osb[:], po[:, cs])
                        eng = nc.sync if _oq[0] % 2 == 0 else nc.gpsimd
                        _oq[0] += 1
                        eng.dma_start(
                            out[r0:r0 + P,
                                ds * QS + pc * pw:ds * QS + (pc + 1) * pw],
                            osb[:]
                        )

        ysbs = {}
        # ---- attention + output projection, per query slice ----
        for j in range(NQS):
            ysb = work.tile([P, HPC, QS], BF, tag="ysb")
            nkb = 4 * j + 4  # causal: key blocks 0..4j+3
            for h in range(HPC):
                yt = pyt.tile([P, QS], F32, tag="yt")
                den = pden.tile([P, QS], F32, tag="den")
                prev_pt = None
                ptot = None
                for tkb in range(nkb):
                    d = tkb - 4 * j
                    # columns left of 128*d are fully masked for diagonal
                    # blocks: skip them in QK/exp/AV; the mask-mult zeroes
                    # the stale region of pt so den/AV sums stay exact.
                    c0 = max(d, 0) * P
                    st = pst.tile([P, QS], F32, tag="st")
                    nc.tensor.matmul(
                        st[:, c0:],
                        KT[:, tkb * P:(tkb + 1) * P],
                        QT[:, h, j * QS + c0:(j + 1) * QS],
                        start=True, stop=True,
                    )
                    pt = ptp.tile([P, QS], BF, tag="pt")
                    nc.scalar.activation(
                        pt[:, c0:], st[:, c0:],
                        mybir.ActivationFunctionType.Exp, scale=SCALE,
                    )
                    if d >= 0:
                        nc.gpsimd.tensor_mul(pt[:], pt[:], mask_sb[:, d, :])
                    nc.tensor.matmul(
                        yt[:, c0:], Vn[:, tkb, :], pt[:, c0:],
                        start=(tkb == 0), stop=(tkb == nkb - 1),
                    )
                    # denominator: tree-sum all PT blocks on DVE, then one
                    # ones-matmul per (h, j) for the partition reduction
                    if tkb % 2 == 0:
                        prev_pt = pt
                    else:
                        pts = ptp.tile([P, QS], BF, tag="pts", name=f"pts{j}_{h}_{tkb}", bufs=4)
                        nc.vector.tensor_add(pts[:], prev_pt[:], pt[:])
                        if tkb % 4 == 1:
                            prev_pts = pts
                        else:
                            ptq = ptp.tile([P, QS], BF, tag="ptq", bufs=4,
                                           name=f"ptq{j}_{h}_{tkb}")
                            nc.vector.tensor_add(ptq[:], prev_pts[:], pts[:])
                            if ptot is None:
                                ptot = ptq
                            else:
                                nxt = ptp.tile([P, QS], BF, tag="ptt", bufs=4,
                                               name=f"ptt{j}_{h}_{tkb}")
                                nc.vector.tensor_add(nxt[:], ptot[:], ptq[:])
                                ptot = nxt
                if True:
                    nc.tensor.matmul(den[:], ones_sb[:], ptot[:],
                                     start=True, stop=True)
                recipb = work.tile([P, QS], F32, tag="recipb", name=f"rb{j}_{h}")
                nc.vector.reciprocal(recipb[:], den[:])
                nc.vector.tensor_mul(ysb[:, h, :], yt[:], recipb[:])

            ysbs[j] = ysb
            if j == 0:
                # PE filler for the exp-bound first attention slice: the
                # last K/V projections aren't needed until attention j>=3.
                proj_k(NQS - 1, cyc=False)
                proj_v_quad(NQS - 1, cyc=False)
            if j >= 1:
                wo_stage(j - 1, ysbs[j - 1])
        wo_stage(NQS - 1, ysbs[NQS - 1], final=True)
    split_multi_waits(nc)
    return nc


def _rope_tables(pos):
    inv_freq = 1.0 / (THETA ** (np.arange(0, DK // 2, dtype=np.float64) * 2.0 / DK))
    ang = pos.astype(np.float64)[:, None] * inv_freq[None, :]   # (T, 64)
    cos = np.cos(ang).T.astype(np.float32)                      # (64, T)
    sin = np.sin(ang).T.astype(np.float32)
    cosf = np.concatenate([cos, cos], axis=0)                   # (128, T)
    sinf = np.concatenate([-sin, sin], axis=0)
    return cosf, sinf


def _make_in_maps(inputs):
    x, Wq, Wk, Wv, Wo = (np.asarray(inputs[k]) for k in
                         ("x", "Wq", "Wk", "Wv", "Wo"))
    bf = ml_dtypes.bfloat16
    cosf, sinf = _rope_tables(np.asarray(inputs["pos"]))
    cosf = cosf.astype(bf)
    sinf = sinf.astype(bf)
    # diagonal-region 0/1 masks: dmask[d][tk, tq] = mask[tq, d*128 + tk]
    m = np.asarray(inputs["mask"])
    dmask = np.stack(
        [m[0:QS, d * P:(d + 1) * P].T for d in range(HPC)], axis=0
    ).astype(bf)

    in_maps = []
    for c in range(8):
        b, g = c // 4, c % 4
        in_maps.append({
            "x": x[b].astype(bf),
            "wq": Wq[:, g * HPC * DK:(g + 1) * HPC * DK].astype(bf),
            "wk": Wk[:, g * DK:(g + 1) * DK].astype(bf),
            "wv": Wv[:, g * DK:(g + 1) * DK].astype(bf),
            "wo": Wo[g * HPC * DK:(g + 1) * HPC * DK, :].astype(bf),
            "cosf": cosf, "sinf": sinf, "dmask": dmask,
        })
    return in_maps


def kernel(x, Wq, Wk, Wv, Wo, mask, pos):
    in_maps = _make_in_maps(dict(x=x, Wq=Wq, Wk=Wk, Wv=Wv, Wo=Wo,
                                 mask=mask, pos=pos))
    if "nc" not in _CACHE:
        _CACHE["nc"] = build_nc()
    nc = _CACHE["nc"]

    res = run_bass_kernel_spmd(nc, in_maps, core_ids=list(range(8)))
    outs = [np.asarray(r["out"], dtype=np.float32) for r in res.results]
    full = np.stack([
        outs[0] + outs[1] + outs[2] + outs[3],
        outs[4] + outs[5] + outs[6] + outs[7],
    ]).astype(np.float32)
    return full



# revision 17
# speedup vs baseline: 1.0086x; 1.0030x over previous
"""Trainium2 Bass kernel for GQA causal attention block (B=2,T=2048,D=2048,H=16,G=4).

Sharding: 8 cores = batch(2) x kv-group(4). Core c handles batch b=c//4 and
kv-group g=c%4 (query heads 4g..4g+3, which share that kv group). Each core
computes a partial output y_g @ Wo[g-rows] for its batch; the host sums the 4
group partials per batch.

Per-core dataflow (all matmul inputs bf16, fp32 PSUM accumulation):
  xT  = dma-transpose(x)                    [d=128, o, t]  (contraction layouts)
  QT_h = wq_h.T @ x.T  (PE, accum over d)   [dk=128, t]
  KT   = wk.T @ x.T                         [dk=128, t]
  V    = x @ wv        (natural)            [t=128-blk, dk]
  RoPE on QT/KT via half-swap (SBUF-SBUF DMA) + mul/mul/add on DVE
  per qslice j (512 queries), head h, key block tkb<=4j+3:
    ST  = KT_blk.T-contraction QK matmul -> PSUM [tk=128, tq=512]
    PT  = exp(scale*ST) on ACT -> SBUF bf16; diag blocks masked by 0/1 mult
    yT += V_blk.T @ PT   (PE accum)          [dk=128, tq=512]
  den = ones128.T @ tree_sum(PT blocks)  (DVE sums, 1 PE matmul) [128, tq=512]
  ysb = yT * recip(den)  (DVE) -> bf16
  out[tq,:] += (partial) sum_h ysb_h.T @ wo_h  (PE accum over heads)
"""

import sys
from contextlib import ExitStack

import numpy as np

sys.path.insert(0, "/opt/trn_rl_repo")

import ml_dtypes

import bass_rust
import concourse.bass as bass
import concourse.mybir as mybir
import concourse.tile as tile
from concourse.bass_utils import run_bass_kernel_spmd

B, T, D = 2, 2048, 2048
H, G, DK = 16, 4, 128
HPC = H // G          # 4 query heads per core
P = 128
NDC = D // P          # 16 contraction chunks
NTB = T // P          # 16 token blocks
QS = 512              # query slice (matmul moving dim)
NQS = T // QS         # 4
ND = D // QS          # 4 output column slices
THETA = 10000.0
SCALE = 1.0 / float(np.sqrt(DK))
BF = mybir.dt.bfloat16
F32 = mybir.dt.float32

_CACHE = {}
_NSPLIT = [0]


def split_multi_waits(nc):
    """Walrus codegen accepts at most one sem wait per instruction; Tile's
    sem assignment can emit several. Hoist extras onto single-wait NOPs
    inserted immediately before, on the same engine stream."""
    n = 0
    for f in nc.m.functions:
        for b in f.blocks:
            insts = b.instructions
            newl = []
            changed = False
            for ins in insts:
                si = getattr(ins, "sync_info", None)
                if si is not None and si.on_wait and len(si.on_wait) > 1:
                    waits = list(si.on_wait)
                    for w in waits[:-1]:
                        _NSPLIT[0] += 1
                        nop = bass_rust.InstNoOp(
                            name=f"I-wsplit{_NSPLIT[0]}",
                            engine=ins.engine,
                            ins=[], outs=[],
                            bass_nofuse=True,
                            sync_info=mybir.SyncInfo(on_wait=[w], on_update=[]),
                        )
                        newl.append(nop)
                        n += 1
                    ins.sync_info = mybir.SyncInfo(
                        on_wait=[waits[-1]], on_update=list(si.on_update or [])
                    )
                    changed = True
                newl.append(ins)
            if changed:
                insts.clear()
                insts.extend(newl)
    return n


def build_nc():
    nc = bass.Bass()
    x = nc.declare_dram_parameter("x", [T, D], BF, isOutput=False)
    wq = nc.declare_dram_parameter("wq", [D, HPC * DK], BF, isOutput=False)
    wk = nc.declare_dram_parameter("wk", [D, DK], BF, isOutput=False)
    wv = nc.declare_dram_parameter("wv", [D, DK], BF, isOutput=False)
    wo = nc.declare_dram_parameter("wo", [HPC * DK, D], BF, isOutput=False)
    cosf = nc.declare_dram_parameter("cosf", [P, T], BF, isOutput=False)
    sinf = nc.declare_dram_parameter("sinf", [P, T], BF, isOutput=False)
    dmask = nc.declare_dram_parameter("dmask", [HPC, P, QS], BF, isOutput=False)
    out = nc.declare_dram_parameter("out", [T, D], BF, isOutput=True)

    with ExitStack() as ctx:
        tc = ctx.enter_context(tile.TileContext(nc))
        const = ctx.enter_context(tc.tile_pool(name="const", bufs=1))
        work = ctx.enter_context(tc.tile_pool(name="work", bufs=3))
        ptp = ctx.enter_context(tc.tile_pool(name="ptp", bufs=8))
        pos_ = ctx.enter_context(tc.tile_pool(name="pos_", bufs=6))
        pst = ctx.enter_context(tc.tile_pool(name="pst", bufs=3, space="PSUM"))
        pyt = ctx.enter_context(tc.tile_pool(name="pyt", bufs=2, space="PSUM"))
        pden = ctx.enter_context(tc.tile_pool(name="pden", bufs=1, space="PSUM"))
        pmm = ctx.enter_context(tc.tile_pool(name="pmm", bufs=2, space="PSUM"))

        # ---- persistent SBUF loads ----
        # Interleave per-chunk loads across the two HWDGE queues (SP carries
        # the xbar transposes, ACT the weight chunks) so the first projection
        # matmuls can start as soon as chunk 0 lands.
        xT = const.tile([P, NDC, T], BF, tag="xT")
        wq_sb = const.tile([P, NDC, HPC * DK], BF, tag="wq")
        wk_sb = const.tile([P, NDC, DK], BF, tag="wk")
        wv_sb = const.tile([P, NDC, DK], BF, tag="wv")
        wq_r = wq.rearrange("(o p) m -> p o m", p=P)
        wk_r = wk.rearrange("(o p) m -> p o m", p=P)
        wv_r = wv.rearrange("(o p) m -> p o m", p=P)
        HT = T // 2
        # first projection matmul needs wq chunk 0 + xT[:, 0, 0:512]: put
        # those two transfers at the head of DIFFERENT queues so they land
        # in parallel (~1.4us instead of ~2.4us serial)
        nc.scalar.dma_start_transpose(xT[:, 0, :QS], x[:QS, 0:P])
        for o in range(NDC):
            nc.sync.dma_start(wq_sb[:, o, :], wq_r[:, o, :])
            if o == 0:
                nc.sync.dma_start_transpose(
                    xT[:, o, QS:HT], x[QS:HT, o * P:(o + 1) * P])
            else:
                nc.sync.dma_start_transpose(
                    xT[:, o, :HT], x[:HT, o * P:(o + 1) * P])
            nc.scalar.dma_start_transpose(
                xT[:, o, HT:], x[HT:, o * P:(o + 1) * P])
            nc.scalar.dma_start(wk_sb[:, o, :], wk_r[:, o, :])
            nc.scalar.dma_start(wv_sb[:, o, :], wv_r[:, o, :])
        wo_sb = const.tile([P, HPC, D], BF, tag="wo")
        nc.scalar.dma_start(wo_sb[:], wo.rearrange("(h p) n -> p h n", p=P))
        cos_sb = const.tile([P, T], BF, tag="cos")
        nc.scalar.dma_start(cos_sb[:], cosf[:])
        sin_sb = const.tile([P, T], BF, tag="sin")
        nc.scalar.dma_start(sin_sb[:], sinf[:])
        mask_sb = const.tile([P, HPC, QS], BF, tag="mask")
        nc.scalar.dma_start(mask_sb[:], dmask.rearrange("d p q -> p d q"))
        ones_sb = const.tile([P, P], BF, tag="ones")
        nc.vector.memset(ones_sb[:], 1.0)
        # zero-init the pt pool slots: diagonal blocks only exp the unmasked
        # columns, and mask*stale-NaN would poison the sums otherwise
        for i in range(8):
            ptz = ptp.tile([P, QS], BF, tag="pt", name=f"ptz{i}")
            nc.vector.memset(ptz[:], 0.0)
        # warm the ACT exp table during the (ACT-idle) projection phase so
        # the 1.28us table load doesn't land on the first attention exp
        warm = work.tile([P, 1], F32, tag="warm", name="warm")
        nc.vector.memset(warm[:], 0.0)
        nc.scalar.activation(warm[:], warm[:],
                             mybir.ActivationFunctionType.Exp)

        # ---- projections (sliced, interleaved with attention) ----
        # Query-slice granularity: Q/K projections + rope are emitted per
        # 512-token slice, V per 4-block quad. Slice 0 runs upfront; slice
        # j+1 is emitted inside attention j as PE filler (the attention
        # inner loop is ACT/exp throughput-bound, so the PE has idle slots).
        _pp = [(pmm, "mm"), (pst, "st"), (pyt, "yt"), (pden, "den")]
        _pg = [0]

        def proj_psum(cyc):
            if cyc:
                pool, tg = _pp[_pg[0] % 4]
                _pg[0] += 1
            else:
                pool, tg = pmm, "mm"
            return pool.tile([P, QS], F32, tag=tg, name=f"pj{_pg[0]}_{cyc}")

        def rope_slice(dst, ts, nm):  # dst: [128, 512] bf16 AP, token slice ts
            sl = slice(ts * QS, (ts + 1) * QS)
            sw = work.tile([P, QS], BF, tag="swp", name=f"sw{nm}")
            nc.gpsimd.dma_start(sw[0:64, :], dst[64:128, :])
            nc.gpsimd.dma_start(sw[64:128, :], dst[0:64, :])
            nc.vector.tensor_mul(sw[:], sw[:], sin_sb[:, sl])
            nc.vector.tensor_mul(dst, dst, cos_sb[:, sl])
            nc.vector.tensor_add(dst, dst, sw[:])

        QT = const.tile([P, HPC, T], BF, tag="QT")
        KT = const.tile([P, T], BF, tag="KT")
        Vn = const.tile([P, NTB, DK], BF, tag="Vn")

        def proj_q(h, ts, cyc=False):
            ps = proj_psum(cyc)
            for o in range(NDC):
                nc.tensor.matmul(
                    ps[:],
                    wq_sb[:, o, h * DK:(h + 1) * DK],
                    xT[:, o, ts * QS:(ts + 1) * QS],
                    start=(o == 0), stop=(o == NDC - 1),
                )
            nc.vector.tensor_copy(QT[:, h, ts * QS:(ts + 1) * QS], ps[:])

        def rope_q4(ts):
            # batched rope over all 4 heads of a query slice: one pair of
            # half-swap DMAs + 3 DVE ops on [128, 4, 512] APs (cos/sin
            # tables broadcast over the head dim with stride 0)
            sl = slice(ts * QS, (ts + 1) * QS)
            qs = QT[:, :, sl]
            sw = work.tile([P, HPC, QS], BF, tag="sw4", name=f"sw4_{ts}", bufs=2)
            nc.gpsimd.dma_start(sw[0:64, :, :], QT[64:128, :, sl])
            nc.gpsimd.dma_start(sw[64:128, :, :], QT[0:64, :, sl])
            sinb = sin_sb[:, sl].rearrange(
                "p (o c) -> p o c", o=1).broadcast_to((P, HPC, QS))
            cosb = cos_sb[:, sl].rearrange(
                "p (o c) -> p o c", o=1).broadcast_to((P, HPC, QS))
            nc.vector.tensor_mul(sw[:], sw[:], sinb)
            nc.vector.tensor_mul(qs, qs, cosb)
            nc.vector.tensor_add(qs, qs, sw[:])

        def proj_k(ts, cyc=False):
            ps = proj_psum(cyc)
            for o in range(NDC):
                nc.tensor.matmul(
                    ps[:], wk_sb[:, o, :], xT[:, o, ts * QS:(ts + 1) * QS],
                    start=(o == 0), stop=(o == NDC - 1),
                )
            nc.vector.tensor_copy(KT[:, ts * QS:(ts + 1) * QS], ps[:])
            rope_slice(KT[:, ts * QS:(ts + 1) * QS], ts, f"k{ts}")

        def proj_v_quad(jq, cyc=False):
            for tb in range(4 * jq, 4 * jq + 4):
                ps = proj_psum(cyc)
                for o in range(NDC):
                    nc.tensor.matmul(
                        ps[:, :DK], xT[:, o, tb * P:(tb + 1) * P], wv_sb[:, o, :],
                        start=(o == 0), stop=(o == NDC - 1),
                    )
                nc.vector.tensor_copy(Vn[:, tb, :], ps[:, :DK])

        for ts in range(NQS):
            for h in range(HPC):
                proj_q(h, ts, cyc=True)
            rope_q4(ts)
        for ts in range(NQS - 1):
            proj_k(ts, cyc=True)
        for jq in range(NQS - 1):
            proj_v_quad(jq, cyc=True)

        _oq = [0]

        def wo_stage(j, ysb, final=False):
            # Final stage: attention PSUM pools are free — cycle po across
            # all four to keep the PE from stalling on bank recycling, and
            # do the PSUM->SBUF copies on the (by then idle) ACT engine.
            pools = _pp if final else [(pmm, "mm")]
            gi = 0
            for tqb in range(QS // P):
                r0 = j * QS + tqb * P
                for ds in range(ND):
                    pool, tg = pools[gi % len(pools)]
                    gi += 1
                    po = pool.tile([P, QS], F32, tag=tg,
                                   name=f"po{j}_{tqb}_{ds}")
                    npc = 1
                    pw = QS // npc
                    for pc in range(npc):
                        cs = slice(pc * pw, (pc + 1) * pw)
                        for h in range(HPC):
                            nc.tensor.matmul(
                                po[:, cs],
                                ysb[:, h, tqb * P:(tqb + 1) * P],
                                wo_sb[:, h, ds * QS + pc * pw:
                                      ds * QS + (pc + 1) * pw],
                                start=(h == 0), stop=(h == HPC - 1),
                            )
                        osb = pos_.tile([P, pw], BF, tag="osb",
                                        name=f"osb{j}_{tqb}_{ds}_{pc}")
                        if final and (gi + pc) % 2 == 0:
                            nc.scalar.copy(osb[:], po[:, cs])
                        else:
                            nc.vector.tensor_copy(osb[:], po[:, cs])
                        eng = nc.sync if _oq[0] % 2 == 0 else nc.gpsimd
                        _oq[0] += 1
                        eng.dma_start(
                            out[r0:r0 + P,
                                ds * QS + pc * pw:ds * QS + (pc + 1) * pw],
                            osb[:]
                        )

        ysbs = {}
        # ---- attention + output projection, per query slice ----
        for j in range(NQS):
            ysb = work.tile([P, HPC, QS], BF, tag="ysb")
            nkb = 4 * j + 4  # causal: key blocks 0..4j+3
            for h in range(HPC):
                yt = pyt.tile([P, QS], F32, tag="yt")
                den = pden.tile([P, QS], F32, tag="den")
                prev_pt = None
                ptot = None
                for tkb in range(nkb):
                    d = tkb - 4 * j
                    # columns left of 128*d are fully masked for diagonal
                    # blocks: skip them in QK/exp/AV; the mask-mult zeroes
                    # the stale region of pt so den/AV sums stay exact.
                    c0 = max(d, 0) * P
                    st = pst.tile([P, QS], F32, tag="st")
                    nc.tensor.matmul(
                        st[:, c0:],
                        KT[:, tkb * P:(tkb + 1) * P],
                        QT[:, h, j * QS + c0:(j + 1) * QS],
                        start=True, stop=True,
                    )
                    pt = ptp.tile([P, QS], BF, tag="pt")
                    nc.scalar.activation(
                        pt[:, c0:], st[:, c0:],
                        mybir.ActivationFunctionType.Exp, scale=SCALE,
                    )
                    if d >= 0:
                        nc.gpsimd.tensor_mul(pt[:], pt[:], mask_sb[:, d, :])
                    nc.tensor.matmul(
                        yt[:, c0:], Vn[:, tkb, :], pt[:, c0:],
                        start=(tkb == 0), stop=(tkb == nkb - 1),
                    )
                    # denominator: tree-sum all PT blocks on DVE, then one
                    # ones-matmul per (h, j) for the partition reduction
                    if tkb % 2 == 0:
                        prev_pt = pt
                    else:
                        pts = ptp.tile([P, QS], BF, tag="pts", name=f"pts{j}_{h}_{tkb}", bufs=4)
                        nc.vector.tensor_add(pts[:], prev_pt[:], pt[:])
                        if tkb % 4 == 1:
                            prev_pts = pts
                        else:
                            ptq = ptp.tile([P, QS], BF, tag="ptq", bufs=4,
                                           name=f"ptq{j}_{h}_{tkb}")
                            nc.vector.tensor_add(ptq[:], prev_pts[:], pts[:])
                            if ptot is None:
                                ptot = ptq
                            else:
                                nxt = ptp.tile([P, QS], BF, tag="ptt", bufs=4,
                                               name=f"ptt{j}_{h}_{tkb}")
                                nc.vector.tensor_add(nxt[:], ptot[:], ptq[:])
                                ptot = nxt
                if True:
                    nc.tensor.matmul(den[:], ones_sb[:], ptot[:],
                                     start=True, stop=True)
                recipb = work.tile([P, QS], F32, tag="recipb", name=f"rb{j}_{h}")
                nc.vector.reciprocal(recipb[:], den[:])
                nc.vector.tensor_mul(ysb[:, h, :], yt[:], recipb[:])

            ysbs[j] = ysb
            if j == 0:
                # PE filler for the exp-bound first attention slice: the
                # last K/V projections aren't needed until attention j>=3.
                proj_k(NQS - 1, cyc=False)
                proj_v_quad(NQS - 1, cyc=False)
            if j >= 1:
                wo_stage(j - 1, ysbs[j - 1])
        wo_stage(NQS - 1, ysbs[NQS - 1], final=True)
    split_multi_waits(nc)
    return nc


def _rope_tables(pos):
    inv_freq = 1.0 / (THETA ** (np.arange(0, DK // 2, dtype=np.float64) * 2.0 / DK))
    ang = pos.astype(np.float64)[:, None] * inv_freq[None, :]   # (T, 64)
    cos = np.cos(ang).T.astype(np.float32)                      # (64, T)
    sin = np.sin(ang).T.astype(np.float32)
    cosf = np.concatenate([cos, cos], axis=0)                   # (128, T)
    sinf = np.concatenate([-sin, sin], axis=0)
    return cosf, sinf


def _make_in_maps(inputs):
    x, Wq, Wk, Wv, Wo = (np.asarray(inputs[k]) for k in
                         ("x", "Wq", "Wk", "Wv", "Wo"))
    bf = ml_dtypes.bfloat16
    cosf, sinf = _rope_tables(np.asarray(inputs["pos"]))
    cosf = cosf.astype(bf)
    sinf = sinf.astype(bf)
    # diagonal-region 0/1 masks: dmask[d][tk, tq] = mask[tq, d*128 + tk]
    m = np.asarray(inputs["mask"])
    dmask = np.stack(
        [m[0:QS, d * P:(d + 1) * P].T for d in range(HPC)], axis=0
    ).astype(bf)

    in_maps = []
    for c in range(8):
        b, g = c // 4, c % 4
        in_maps.append({
            "x": x[b].astype(bf),
            "wq": Wq[:, g * HPC * DK:(g + 1) * HPC * DK].astype(bf),
            "wk": Wk[:, g * DK:(g + 1) * DK].astype(bf),
            "wv": Wv[:, g * DK:(g + 1) * DK].astype(bf),
            "wo": Wo[g * HPC * DK:(g + 1) * HPC * DK, :].astype(bf),
            "cosf": cosf, "sinf": sinf, "dmask": dmask,
        })
    return in_maps


def kernel(x, Wq, Wk, Wv, Wo, mask, pos):
    in_maps = _make_in_maps(dict(x=x, Wq=Wq, Wk=Wk, Wv=Wv, Wo=Wo,
                                 mask=mask, pos=pos))
    if "nc" not in _CACHE:
        _CACHE["nc"] = build_nc()
    nc = _CACHE["nc"]

    res = run_bass_kernel_spmd(nc, in_maps, core_ids=list(range(8)))
    outs = [np.asarray(r["out"], dtype=np.float32) for r in res.results]
    full = np.stack([
        outs[0] + outs[1] + outs[2] + outs[3],
        outs[4] + outs[5] + outs[6] + outs[7],
    ]).astype(np.float32)
    return full



# revision 19
# speedup vs baseline: 1.0105x; 1.0018x over previous
"""Trainium2 Bass kernel for GQA causal attention block (B=2,T=2048,D=2048,H=16,G=4).

Sharding: 8 cores = batch(2) x kv-group(4). Core c handles batch b=c//4 and
kv-group g=c%4 (query heads 4g..4g+3, which share that kv group). Each core
computes a partial output y_g @ Wo[g-rows] for its batch; the host sums the 4
group partials per batch.

Per-core dataflow (all matmul inputs bf16, fp32 PSUM accumulation):
  xT  = dma-transpose(x)                    [d=128, o, t]  (contraction layouts)
  QT_h = wq_h.T @ x.T  (PE, accum over d)   [dk=128, t]
  KT   = wk.T @ x.T                         [dk=128, t]
  V    = x @ wv        (natural)            [t=128-blk, dk]
  RoPE on QT/KT via half-swap (SBUF-SBUF DMA) + mul/mul/add on DVE
  per qslice j (512 queries), head h, key block tkb<=4j+3:
    ST  = KT_blk.T-contraction QK matmul -> PSUM [tk=128, tq=512]
    PT  = exp(scale*ST) on ACT -> SBUF bf16; diag blocks masked by 0/1 mult
    yT += V_blk.T @ PT   (PE accum)          [dk=128, tq=512]
  den = ones128.T @ tree_sum(PT blocks)  (DVE sums, 1 PE matmul) [128, tq=512]
  ysb = yT * recip(den)  (DVE) -> bf16
  out[tq,:] += (partial) sum_h ysb_h.T @ wo_h  (PE accum over heads)
"""

import sys
from contextlib import ExitStack

import numpy as np

sys.path.insert(0, "/opt/trn_rl_repo")

import ml_dtypes

import bass_rust
import concourse.bass as bass
import concourse.mybir as mybir
import concourse.tile as tile
from concourse.bass_utils import run_bass_kernel_spmd

B, T, D = 2, 2048, 2048
H, G, DK = 16, 4, 128
HPC = H // G          # 4 query heads per core
P = 128
NDC = D // P          # 16 contraction chunks
NTB = T // P          # 16 token blocks
QS = 512              # query slice (matmul moving dim)
NQS = T // QS         # 4
ND = D // QS          # 4 output column slices
THETA = 10000.0
SCALE = 1.0 / float(np.sqrt(DK))
BF = mybir.dt.bfloat16
F32 = mybir.dt.float32

_CACHE = {}
_NSPLIT = [0]


def split_multi_waits(nc):
    """Walrus codegen accepts at most one sem wait per instruction; Tile's
    sem assignment can emit several. Hoist extras onto single-wait NOPs
    inserted immediately before, on the same engine stream."""
    n = 0
    for f in nc.m.functions:
        for b in f.blocks:
            insts = b.instructions
            newl = []
            changed = False
            for ins in insts:
                si = getattr(ins, "sync_info", None)
                if si is not None and si.on_wait and len(si.on_wait) > 1:
                    waits = list(si.on_wait)
                    for w in waits[:-1]:
                        _NSPLIT[0] += 1
                        nop = bass_rust.InstNoOp(
                            name=f"I-wsplit{_NSPLIT[0]}",
                            engine=ins.engine,
                            ins=[], outs=[],
                            bass_nofuse=True,
                            sync_info=mybir.SyncInfo(on_wait=[w], on_update=[]),
                        )
                        newl.append(nop)
                        n += 1
                    ins.sync_info = mybir.SyncInfo(
                        on_wait=[waits[-1]], on_update=list(si.on_update or [])
                    )
                    changed = True
                newl.append(ins)
            if changed:
                insts.clear()
                insts.extend(newl)
    return n


def build_nc():
    nc = bass.Bass()
    x = nc.declare_dram_parameter("x", [T, D], BF, isOutput=False)
    wq = nc.declare_dram_parameter("wq", [D, HPC * DK], BF, isOutput=False)
    wk = nc.declare_dram_parameter("wk", [D, DK], BF, isOutput=False)
    wv = nc.declare_dram_parameter("wv", [D, DK], BF, isOutput=False)
    wo = nc.declare_dram_parameter("wo", [HPC * DK, D], BF, isOutput=False)
    cosf = nc.declare_dram_parameter("cosf", [P, T], BF, isOutput=False)
    sinf = nc.declare_dram_parameter("sinf", [P, T], BF, isOutput=False)
    dmask = nc.declare_dram_parameter("dmask", [HPC, P, QS], BF, isOutput=False)
    out = nc.declare_dram_parameter("out", [T, D], BF, isOutput=True)

    with ExitStack() as ctx:
        tc = ctx.enter_context(tile.TileContext(nc))
        const = ctx.enter_context(tc.tile_pool(name="const", bufs=1))
        work = ctx.enter_context(tc.tile_pool(name="work", bufs=3))
        ptp = ctx.enter_context(tc.tile_pool(name="ptp", bufs=8))
        pos_ = ctx.enter_context(tc.tile_pool(name="pos_", bufs=6))
        pst = ctx.enter_context(tc.tile_pool(name="pst", bufs=3, space="PSUM"))
        pyt = ctx.enter_context(tc.tile_pool(name="pyt", bufs=2, space="PSUM"))
        pden = ctx.enter_context(tc.tile_pool(name="pden", bufs=1, space="PSUM"))
        pmm = ctx.enter_context(tc.tile_pool(name="pmm", bufs=2, space="PSUM"))

        # ---- persistent SBUF loads ----
        # Interleave per-chunk loads across the two HWDGE queues (SP carries
        # the xbar transposes, ACT the weight chunks) so the first projection
        # matmuls can start as soon as chunk 0 lands.
        xT = const.tile([P, NDC, T], BF, tag="xT")
        wq_sb = const.tile([P, NDC, HPC * DK], BF, tag="wq")
        wk_sb = const.tile([P, NDC, DK], BF, tag="wk")
        wv_sb = const.tile([P, NDC, DK], BF, tag="wv")
        wq_r = wq.rearrange("(o p) m -> p o m", p=P)
        wk_r = wk.rearrange("(o p) m -> p o m", p=P)
        wv_r = wv.rearrange("(o p) m -> p o m", p=P)
        HT = T // 2
        # first projection matmul needs wq chunk 0 + xT[:, 0, 0:512]: put
        # those two transfers at the head of DIFFERENT queues so they land
        # in parallel (~1.4us instead of ~2.4us serial)
        nc.scalar.dma_start_transpose(xT[:, 0, :QS], x[:QS, 0:P])
        for o in range(NDC):
            nc.sync.dma_start(wq_sb[:, o, :], wq_r[:, o, :])
            if o == 0:
                nc.sync.dma_start_transpose(
                    xT[:, o, QS:HT], x[QS:HT, o * P:(o + 1) * P])
            else:
                nc.sync.dma_start_transpose(
                    xT[:, o, :HT], x[:HT, o * P:(o + 1) * P])
            nc.scalar.dma_start_transpose(
                xT[:, o, HT:], x[HT:, o * P:(o + 1) * P])
            nc.scalar.dma_start(wk_sb[:, o, :], wk_r[:, o, :])
            nc.scalar.dma_start(wv_sb[:, o, :], wv_r[:, o, :])
        wo_sb = const.tile([P, HPC, D], BF, tag="wo")
        nc.scalar.dma_start(wo_sb[:], wo.rearrange("(h p) n -> p h n", p=P))
        cos_sb = const.tile([P, T], BF, tag="cos")
        nc.scalar.dma_start(cos_sb[:], cosf[:])
        sin_sb = const.tile([P, T], BF, tag="sin")
        nc.scalar.dma_start(sin_sb[:], sinf[:])
        mask_sb = const.tile([P, HPC, QS], BF, tag="mask")
        nc.scalar.dma_start(mask_sb[:], dmask.rearrange("d p q -> p d q"))
        ones_sb = const.tile([P, P], BF, tag="ones")
        nc.vector.memset(ones_sb[:], 1.0)
        # zero-init the pt pool slots: diagonal blocks only exp the unmasked
        # columns, and mask*stale-NaN would poison the sums otherwise
        for i in range(8):
            ptz = ptp.tile([P, QS], BF, tag="pt", name=f"ptz{i}")
            nc.vector.memset(ptz[:], 0.0)
        # warm the ACT exp table during the (ACT-idle) projection phase so
        # the 1.28us table load doesn't land on the first attention exp
        warm = work.tile([P, 1], F32, tag="warm", name="warm")
        nc.vector.memset(warm[:], 0.0)
        nc.scalar.activation(warm[:], warm[:],
                             mybir.ActivationFunctionType.Exp)

        # ---- projections (sliced, interleaved with attention) ----
        # Query-slice granularity: Q/K projections + rope are emitted per
        # 512-token slice, V per 4-block quad. Slice 0 runs upfront; slice
        # j+1 is emitted inside attention j as PE filler (the attention
        # inner loop is ACT/exp throughput-bound, so the PE has idle slots).
        _pp = [(pmm, "mm"), (pst, "st"), (pyt, "yt"), (pden, "den")]
        _pg = [0]

        def proj_psum(cyc):
            if cyc:
                pool, tg = _pp[_pg[0] % 4]
                _pg[0] += 1
            else:
                pool, tg = pmm, "mm"
            return pool.tile([P, QS], F32, tag=tg, name=f"pj{_pg[0]}_{cyc}")

        def rope_slice(dst, ts, nm):  # dst: [128, 512] bf16 AP, token slice ts
            sl = slice(ts * QS, (ts + 1) * QS)
            sw = work.tile([P, QS], BF, tag="swp", name=f"sw{nm}")
            nc.gpsimd.dma_start(sw[0:64, :], dst[64:128, :])
            nc.gpsimd.dma_start(sw[64:128, :], dst[0:64, :])
            nc.vector.tensor_mul(sw[:], sw[:], sin_sb[:, sl])
            nc.vector.tensor_mul(dst, dst, cos_sb[:, sl])
            nc.vector.tensor_add(dst, dst, sw[:])

        QT = const.tile([P, HPC, T], BF, tag="QT")
        KT = const.tile([P, T], BF, tag="KT")
        Vn = const.tile([P, NTB, DK], BF, tag="Vn")

        def proj_q(h, ts, cyc=False):
            ps = proj_psum(cyc)
            for o in range(NDC):
                nc.tensor.matmul(
                    ps[:],
                    wq_sb[:, o, h * DK:(h + 1) * DK],
                    xT[:, o, ts * QS:(ts + 1) * QS],
                    start=(o == 0), stop=(o == NDC - 1),
                )
            nc.vector.tensor_copy(QT[:, h, ts * QS:(ts + 1) * QS], ps[:])

        def rope_q4(ts):
            # batched rope over all 4 heads of a query slice: one pair of
            # half-swap DMAs + 3 DVE ops on [128, 4, 512] APs (cos/sin
            # tables broadcast over the head dim with stride 0)
            sl = slice(ts * QS, (ts + 1) * QS)
            qs = QT[:, :, sl]
            sw = work.tile([P, HPC, QS], BF, tag="sw4", name=f"sw4_{ts}", bufs=2)
            nc.gpsimd.dma_start(sw[0:64, :, :], QT[64:128, :, sl])
            nc.gpsimd.dma_start(sw[64:128, :, :], QT[0:64, :, sl])
            sinb = sin_sb[:, sl].rearrange(
                "p (o c) -> p o c", o=1).broadcast_to((P, HPC, QS))
            cosb = cos_sb[:, sl].rearrange(
                "p (o c) -> p o c", o=1).broadcast_to((P, HPC, QS))
            nc.vector.tensor_mul(sw[:], sw[:], sinb)
            nc.vector.tensor_mul(qs, qs, cosb)
            nc.vector.tensor_add(qs, qs, sw[:])

        def proj_k(ts, cyc=False):
            ps = proj_psum(cyc)
            for o in range(NDC):
                nc.tensor.matmul(
                    ps[:], wk_sb[:, o, :], xT[:, o, ts * QS:(ts + 1) * QS],
                    start=(o == 0), stop=(o == NDC - 1),
                )
            nc.vector.tensor_copy(KT[:, ts * QS:(ts + 1) * QS], ps[:])
            rope_slice(KT[:, ts * QS:(ts + 1) * QS], ts, f"k{ts}")

        def proj_v_quad(jq, cyc=False):
            for tb in range(4 * jq, 4 * jq + 4):
                ps = proj_psum(cyc)
                for o in range(NDC):
                    nc.tensor.matmul(
                        ps[:, :DK], xT[:, o, tb * P:(tb + 1) * P], wv_sb[:, o, :],
                        start=(o == 0), stop=(o == NDC - 1),
                    )
                nc.vector.tensor_copy(Vn[:, tb, :], ps[:, :DK])

        for ts in range(NQS):
            for h in range(HPC):
                proj_q(h, ts, cyc=True)
            rope_q4(ts)
        for ts in range(NQS - 1):
            proj_k(ts, cyc=True)
        for jq in range(NQS - 1):
            proj_v_quad(jq, cyc=True)

        _oq = [0]

        def wo_stage(j, ysb, final=False):
            # Final stage: attention PSUM pools are free — cycle po across
            # all four to keep the PE from stalling on bank recycling, and
            # do the PSUM->SBUF copies on the (by then idle) ACT engine.
            pools = _pp if final else [(pmm, "mm")]
            gi = 0
            for tqb in range(QS // P):
                r0 = j * QS + tqb * P
                for ds in range(ND):
                    pool, tg = pools[gi % len(pools)]
                    gi += 1
                    po = pool.tile([P, QS], F32, tag=tg,
                                   name=f"po{j}_{tqb}_{ds}")
                    # split the very last group into two half-width groups
                    # on DIFFERENT psum banks so the final copy+DMA drain
                    # pipelines instead of serializing after the last matmul
                    last = final and tqb == QS // P - 1 and ds == ND - 1
                    npc = 2 if last else 1
                    pw = QS // npc
                    for pc in range(npc):
                        if pc > 0:
                            pool, tg = pools[gi % len(pools)]
                            gi += 1
                            po = pool.tile([P, QS], F32, tag=tg,
                                           name=f"po{j}_{tqb}_{ds}_{pc}")
                        cs = slice(0, pw) if npc > 1 else slice(pc * pw,
                                                                (pc + 1) * pw)
                        for h in range(HPC):
                            nc.tensor.matmul(
                                po[:, cs],
                                ysb[:, h, tqb * P:(tqb + 1) * P],
                                wo_sb[:, h, ds * QS + pc * pw:
                                      ds * QS + (pc + 1) * pw],
                                start=(h == 0), stop=(h == HPC - 1),
                            )
                        osb = pos_.tile([P, QS], BF, tag="osb",
                                        name=f"osb{j}_{tqb}_{ds}_{pc}")
                        osb = osb[:, :pw]
                        if final and (gi + pc) % 2 == 0:
                            nc.scalar.copy(osb[:], po[:, cs])
                        else:
                            nc.vector.tensor_copy(osb[:], po[:, cs])
                        eng = nc.sync if _oq[0] % 2 == 0 else nc.gpsimd
                        _oq[0] += 1
                        eng.dma_start(
                            out[r0:r0 + P,
                                ds * QS + pc * pw:ds * QS + (pc + 1) * pw],
                            osb[:]
                        )

        ysbs = {}
        # ---- attention + output projection, per query slice ----
        for j in range(NQS):
            ysb = work.tile([P, HPC, QS], BF, tag="ysb")
            nkb = 4 * j + 4  # causal: key blocks 0..4j+3
            for h in range(HPC):
                yt = pyt.tile([P, QS], F32, tag="yt")
                den = pden.tile([P, QS], F32, tag="den")
                prev_pt = None
                ptot = None
                for tkb in range(nkb):
                    d = tkb - 4 * j
                    # columns left of 128*d are fully masked for diagonal
                    # blocks: skip them in QK/exp/AV; the mask-mult zeroes
                    # the stale region of pt so den/AV sums stay exact.
                    c0 = max(d, 0) * P
                    st = pst.tile([P, QS], F32, tag="st")
                    nc.tensor.matmul(
                        st[:, c0:],
                        KT[:, tkb * P:(tkb + 1) * P],
                        QT[:, h, j * QS + c0:(j + 1) * QS],
                        start=True, stop=True,
                    )
                    pt = ptp.tile([P, QS], BF, tag="pt")
                    nc.scalar.activation(
                        pt[:, c0:], st[:, c0:],
                        mybir.ActivationFunctionType.Exp, scale=SCALE,
                    )
                    if d >= 0:
                        nc.gpsimd.tensor_mul(pt[:], pt[:], mask_sb[:, d, :])
                    nc.tensor.matmul(
                        yt[:, c0:], Vn[:, tkb, :], pt[:, c0:],
                        start=(tkb == 0), stop=(tkb == nkb - 1),
                    )
                    # denominator: tree-sum all PT blocks on DVE, then one
                    # ones-matmul per (h, j) for the partition reduction
                    if tkb % 2 == 0:
                        prev_pt = pt
                    else:
                        pts = ptp.tile([P, QS], BF, tag="pts", name=f"pts{j}_{h}_{tkb}", bufs=4)
                        nc.vector.tensor_add(pts[:], prev_pt[:], pt[:])
                        if tkb % 4 == 1:
                            prev_pts = pts
                        else:
                            ptq = ptp.tile([P, QS], BF, tag="ptq", bufs=4,
                                           name=f"ptq{j}_{h}_{tkb}")
                            nc.vector.tensor_add(ptq[:], prev_pts[:], pts[:])
                            if ptot is None:
                                ptot = ptq
                            else:
                                nxt = ptp.tile([P, QS], BF, tag="ptt", bufs=4,
                                               name=f"ptt{j}_{h}_{tkb}")
                                nc.vector.tensor_add(nxt[:], ptot[:], ptq[:])
                                ptot = nxt
                if True:
                    nc.tensor.matmul(den[:], ones_sb[:], ptot[:],
                                     start=True, stop=True)
                recipb = work.tile([P, QS], F32, tag="recipb", name=f"rb{j}_{h}")
                nc.vector.reciprocal(recipb[:], den[:])
                nc.vector.tensor_mul(ysb[:, h, :], yt[:], recipb[:])

            ysbs[j] = ysb
            if j == 0:
                # PE filler for the exp-bound first attention slice: the
                # last K/V projections aren't needed until attention j>=3.
                proj_k(NQS - 1, cyc=False)
                proj_v_quad(NQS - 1, cyc=False)
            if j >= 1:
                wo_stage(j - 1, ysbs[j - 1])
        wo_stage(NQS - 1, ysbs[NQS - 1], final=True)
    split_multi_waits(nc)
    return nc


def _rope_tables(pos):
    inv_freq = 1.0 / (THETA ** (np.arange(0, DK // 2, dtype=np.float64) * 2.0 / DK))
    ang = pos.astype(np.float64)[:, None] * inv_freq[None, :]   # (T, 64)
    cos = np.cos(ang).T.astype(np.float32)                      # (64, T)
    sin = np.sin(ang).T.astype(np.float32)
    cosf = np.concatenate([cos, cos], axis=0)                   # (128, T)
    sinf = np.concatenate([-sin, sin], axis=0)
    return cosf, sinf


def _make_in_maps(inputs):
    x, Wq, Wk, Wv, Wo = (np.asarray(inputs[k]) for k in
                         ("x", "Wq", "Wk", "Wv", "Wo"))
    bf = ml_dtypes.bfloat16
    cosf, sinf = _rope_tables(np.asarray(inputs["pos"]))
    cosf = cosf.astype(bf)
    sinf = sinf.astype(bf)
    # diagonal-region 0/1 masks: dmask[d][tk, tq] = mask[tq, d*128 + tk]
    m = np.asarray(inputs["mask"])
    dmask = np.stack(
        [m[0:QS, d * P:(d + 1) * P].T for d in range(HPC)], axis=0
    ).astype(bf)

    in_maps = []
    for c in range(8):
        b, g = c // 4, c % 4
        in_maps.append({
            "x": x[b].astype(bf),
            "wq": Wq[:, g * HPC * DK:(g + 1) * HPC * DK].astype(bf),
            "wk": Wk[:, g * DK:(g + 1) * DK].astype(bf),
            "wv": Wv[:, g * DK:(g + 1) * DK].astype(bf),
            "wo": Wo[g * HPC * DK:(g + 1) * HPC * DK, :].astype(bf),
            "cosf": cosf, "sinf": sinf, "dmask": dmask,
        })
    return in_maps


def kernel(x, Wq, Wk, Wv, Wo, mask, pos):
    in_maps = _make_in_maps(dict(x=x, Wq=Wq, Wk=Wk, Wv=Wv, Wo=Wo,
                                 mask=mask, pos=pos))
    if "nc" not in _CACHE:
        _CACHE["nc"] = build_nc()
    nc = _CACHE["nc"]

    res = run_bass_kernel_spmd(nc, in_maps, core_ids=list(range(8)))
    outs = [np.asarray(r["out"], dtype=np.float32) for r in res.results]
    full = np.stack([
        outs[0] + outs[1] + outs[2] + outs[3],
        outs[4] + outs[5] + outs[6] + outs[7],
    ]).astype(np.float32)
    return full

